# revision 51
# baseline (speedup 1.0000x reference)
"""Causal self-attention (GPT-2 style) Bass kernel for Trainium2.

B=8, T=1024, C=768, NH=12, HD=64. Data-parallel over batch: each of the 8
NeuronCores computes one batch element end to end.

Per-core plan (all matmul inputs bf16, fp32 PSUM accumulation):
  - x DGE-cast to bf16 on load; x^T via bf16 PE transposes, ti-major so
    transposes start as soon as the first x tile lands.
  - W_attn DMA'd in q/k/v column blocks so q/k projections start when the
    first third of the weights has arrived; heads emitted in pair order
    (q0,k0,q1,k1,...) so attention for pair 0 overlaps the rest of QKV.
  - S^T tiles = k^T.T @ q^T per head, restricted to causally-live query
    columns (width 512 - max(0, 128*jc - 512*ic)); the 12 variable-width
    tiles pack exactly into 3 groups of [128, 1536] per head. Head A and
    head B of a pair write the two halves of one [128, 3072] PSUM tile
    (PE rows 0-63 / 64-127 interleaved for row-group concurrency).
  - one exp (ScalarE) per [128, 3072] group-pair -> bf16 P^T slab; the
    intra-tile causal triangles are zeroed post-exp by 5 batched DVE
    multiplies per pair against one [128,128] lower-triangular mask.
  - y'^T = v_aug.T @ P^T: M=65 matmul gives y' rows 0-63 and the softmax
    denominator row 64 for free, accumulating causally-live windows only.
  - denominators broadcast via K=1 rank-1 PE matmuls into a PSUM tile;
    1/l = exp(-ln l) on ScalarE; y normalized by one DVE mul per window.
  - out = y @ W_proj + b via K=128 matmuls from the transposed y layout;
    biases ride K=1 ones matmuls; PSUM pools laid out so attention/proj
    overlap the neighbouring phases.
"""
import numpy as np

import bass_rust
import concourse.bass as bass
import concourse.mybir as mybir
import concourse.tile as tile
from concourse.bass_utils import run_bass_kernel_spmd
from concourse.masks import make_identity
from concourse.vector_clock import ScopedClock

F32 = mybir.dt.float32
BF16 = mybir.dt.bfloat16
AF = mybir.ActivationFunctionType

B, T, C, NH, HD = 8, 1024, 768, 12, 64
C3 = 3 * C
SCALE = 1.0 / 8.0  # 1/sqrt(HD)

# --- causal tiling tables -------------------------------------------------
# S^T tile (jc, ic): keys [128jc, 128jc+128), queries [512ic, 512ic+512).
# Fully-masked query columns (q < 128jc) are trimmed from the left; the
# surviving width is 512 - trim with trim = max(0, 128jc - 512ic).
VALID = [(jc, ic) for ic in (0, 1) for jc in range(8) if 128 * jc < 512 * (ic + 1)]


def _trim(jc, ic):
    return max(0, 128 * jc - 512 * ic)


# Packing of the causally-live columns into 9 groups of exactly 512 per
# head: each head's half of the paired PSUM tile is one full bank, so
# concurrent A/B matmuls never share a bank, and [128,1024] tiles allow
# bufs=3 (triple buffering — no S-fill stall behind the exp reads).
# Entries are (jc, ic, group_offset, width, src_off).
GROUPS_LAYOUT = [
    [(0, 0, 0, 512, 0)],
    [(0, 1, 0, 512, 0)],
    [(1, 1, 0, 512, 0)],
    [(2, 1, 0, 512, 0)],
    [(3, 1, 0, 512, 0)],
    [(4, 1, 0, 512, 0)],
    [(1, 0, 0, 384, 0), (3, 0, 384, 128, 0)],
    [(5, 1, 0, 384, 0), (7, 1, 384, 128, 0)],
    [(2, 0, 0, 256, 0), (6, 1, 256, 256, 0)],
]
# slab fragments per (jc, ic): list of (group, group_off, width, src_off)
TILE_POS = {}
for _g, _grp in enumerate(GROUPS_LAYOUT):
    for (_jc, _ic, _off, _w, _so) in _grp:
        TILE_POS.setdefault((_jc, _ic), []).append((_g, _off, _w, _so))
GW = 512           # per-head group width
PW = 2 * GW        # paired group width (A cols 0-512, B cols 512-1024)
SLABW = 9 * PW     # per-pair slab width (9216)


class TileContextFixed(tile.TileContext):
    """Splits sem waits beyond walrus's per-instruction cap onto NOPs/Drains."""

    def _split_excess_waits(self, inst):
        si = inst.sync_info
        if si is None or not si.on_wait:
            return []
        cap = 2 if isinstance(inst, mybir.InstEventSemaphore) else 1
        waits = list(si.on_wait)
        if len(waits) <= cap:
            return []
        keep = waits[len(waits) - cap:]
        excess = waits[: len(waits) - cap]
        inst.sync_info = bass_rust.SyncInfo(
            on_wait=keep, on_update=list(si.on_update or [])
        )
        nops = []
        for w in excess:
            nop = mybir.InstNoOp(name=f"I-wsplit-{self.nc.next_id()}")
            nop.engine = inst.engine
            nop.sync_info = bass_rust.SyncInfo(on_wait=[w], on_update=[])
            nops.append(nop)
        return nops

    def _commit_instruction(self, inst, lazy_reg_writes: bool = True):
        for nop in self._split_excess_waits(inst):
            self._add_instruction(nop)
        super()._commit_instruction(inst, lazy_reg_writes)

    def _drain_and_barrier(self, tick_clock, wait_clock):
        drain_inst = self.nc.sync.drain()
        wait_clock.add_sem_waits(
            drain_inst.ins, ScopedClock({None: tick_clock.global_clock})
        )
        si = drain_inst.ins.sync_info
        if si is not None and si.on_wait and len(si.on_wait) > 1:
            waits = list(si.on_wait)
            ups = list(si.on_update) if si.on_update else []
            drain_inst.ins.sync_info = bass_rust.SyncInfo(
                on_wait=[waits[0]], on_update=[]
            )
            for i, w in enumerate(waits[1:]):
                d2 = self.nc.sync.drain()
                d2.ins.sync_info = bass_rust.SyncInfo(
                    on_wait=[w], on_update=ups if i == len(waits) - 2 else []
                )
        self.nc.all_engine_barrier()
        assert self.sems is not None
        popped = self.nc._tile_sem_poison_stack.pop()
        assert popped is self._sem_poison
        self.nc.clear_and_free_semaphores(list(self.sems.allocated().values()))
        self.nc.all_engine_barrier()


def build_nc():
    nc = bass.Bass()
    X = nc.declare_dram_parameter("x", [T, C], F32, isOutput=False)
    WA = nc.declare_dram_parameter("W_attn", [C, C3], F32, isOutput=False)
    BA = nc.declare_dram_parameter("b_attn", [C3], F32, isOutput=False)
    WP = nc.declare_dram_parameter("W_proj", [C, C], F32, isOutput=False)
    BP = nc.declare_dram_parameter("b_proj", [C], F32, isOutput=False)
    OUT = nc.declare_dram_parameter("out", [T, C], F32, isOutput=True)

    with TileContextFixed(nc) as tc:
        with tc.tile_pool(name="const", bufs=1) as const:
            # ---- x loads first on the sync HWDGE queue (fast), so the
            # slow SWDGE track is free for the even W-chunk casts ----
            xnat = []
            with tc.tile_pool(name="stage", bufs=1) as stage:
                for ti in range(8):
                    xf = stage.tile([128, C], F32, tag=f"xstage{ti}", bufs=1)
                    nc.sync.dma_start(out=xf, in_=X.ap()[128 * ti: 128 * (ti + 1), :])
                    xnat.append(xf)

                # ---- small constants ----
                b_attn_col = const.tile([128, 18], F32)
                nc.sync.dma_start(
                    out=b_attn_col, in_=BA.ap().rearrange("(a p) -> p a", p=128)
                )
                b_attn_bf = const.tile([1, C3], BF16)
                nc.gpsimd.dma_start(
                    out=b_attn_bf, in_=BA.ap().rearrange("(a c) -> a c", a=1)
                )
                b_proj_bf = const.tile([1, C], BF16)
                nc.gpsimd.dma_start(
                    out=b_proj_bf, in_=BP.ap().rearrange("(a c) -> a c", a=1)
                )
                ones_row = const.tile([1, 128], BF16)
                nc.vector.memset(ones_row, 1.0)
                ident_f = const.tile([128, 128], F32)
                make_identity(nc, ident_f)
                # M0[j, c] = 1 if c >= j else 0 (lower-shift keep mask)
                m0f = const.tile([128, 128], F32)
                nc.vector.memset(m0f, 1.0)
                nc.gpsimd.affine_select(
                    out=m0f,
                    in_=m0f,
                    compare_op=mybir.AluOpType.is_ge,
                    fill=0.0,
                    base=0,
                    pattern=[[1, 128]],
                    channel_multiplier=-1,
                )
                m0 = const.tile([128, 128], BF16)
                nc.vector.tensor_copy(m0, m0f)

                # ---- weights, ordered for earliest attention start:
                # q+k cols for pairs 0-2 first, then pairs 3-5, then the v
                # block, then W_proj (needed last). Even chunks DGE-cast on
                # gpsimd; odd chunks staged via the two HWDGE queues. ----
                w_attn_bf = [
                    const.tile([128, C3], BF16, name=f"wab{c}") for c in range(6)
                ]
                with tc.tile_pool(name="wstage", bufs=1) as wst:
                    # All odd-chunk staged DMA ISSUES go out early on the
                    # scalar HWDGE queue (sync is busy with x); their bf16
                    # casts are deferred until after the xT copies so the
                    # DVE queue never blocks the critical qk path.
                    deferred_casts = []
                    nstg = 0

                    def stage_load(eng, c, cols):
                        nonlocal nstg
                        stg = wst.tile(
                            [128, C], F32, tag=f"ws{nstg}", bufs=1, name=f"ws{nstg}"
                        )
                        nstg += 1
                        eng.dma_start(
                            out=stg[:, 0: cols.stop - cols.start],
                            in_=WA.ap()[128 * c: 128 * (c + 1), cols],
                        )
                        deferred_casts.append((c, cols, stg))

                    # full-width q/k/v column blocks: even chunks SWDGE-cast
                    # on gpsimd, odd chunks staged fp32 on the scalar HWDGE
                    # queue (q/k) or behind x on sync (v); casts deferred.
                    for blk in range(3):
                        cols = slice(C * blk, C * (blk + 1))
                        for c in range(6):
                            if c % 2 == 0:
                                nc.gpsimd.dma_start(
                                    out=w_attn_bf[c][:, cols],
                                    in_=WA.ap()[128 * c: 128 * (c + 1), cols],
                                )
                            else:
                                stage_load(
                                    nc.sync if blk == 2 else nc.scalar, c, cols
                                )
                    w_proj_bf = []
                    for c in range(6):
                        w = const.tile([128, C], BF16, name=f"wpb{c}")
                        nc.gpsimd.dma_start(
                            out=w, in_=WP.ap()[128 * c: 128 * (c + 1), :]
                        )
                        w_proj_bf.append(w)

                    # ---- x^T via PE transposes (fp32 in, bf16 out) ----
                    xT = [const.tile([128, T], BF16, name=f"xT{c}") for c in range(6)]
                    with tc.tile_pool(name="xps", bufs=6, space="PSUM") as xps:
                        for tg in range(2):
                            tps = [
                                xps.tile([128, 512], F32, tag="xps", name=f"tp{c}_{tg}")
                                for c in range(6)
                            ]
                            for q in range(4):
                                ti = 4 * tg + q
                                for c in range(6):
                                    nc.tensor.transpose(
                                        tps[c][:, 128 * q: 128 * (q + 1)],
                                        xnat[ti][:, 128 * c: 128 * (c + 1)],
                                        ident_f,
                                    )
                            for c in range(6):
                                if (c + tg) % 2:
                                    nc.scalar.copy(
                                        xT[c][:, 512 * tg: 512 * (tg + 1)], tps[c]
                                    )
                                else:
                                    nc.vector.tensor_copy(
                                        xT[c][:, 512 * tg: 512 * (tg + 1)], tps[c]
                                    )

                    # deferred W casts (DVE), in load order
                    for (c, cols, stg) in deferred_casts:
                        nc.vector.tensor_copy(
                            w_attn_bf[c][:, cols], stg[:, 0: cols.stop - cols.start]
                        )

            # ---- persistent activations ----
            # v_aug padded to 128 weight columns per head (cols 65-127 zero)
            # so the AV LDWEIGHTS qualifies for fast-weight-load.
            qT = [const.tile([128, T], BF16, name=f"qT{i}") for i in range(6)]
            kT = [const.tile([128, T], BF16, name=f"kT{i}") for i in range(6)]
            v_aug = [
                const.tile([128, NH, 2 * HD], BF16, name=f"vau{i}") for i in range(8)
            ]
            y_pair = [const.tile([128, T], BF16, name=f"yp{i}") for i in range(6)]

            # Fused qkv + attention, software-pipelined two pairs deep:
            # block p emits pair p's q/k projections, pair p's S/exp groups,
            # pair p-2's AV tiles (interleaved among the S groups so the
            # in-order tensor queue always has ready work), and pair p-2's
            # normalization. PSUM: spair 3x[128,1024] (banks 0-5) + one
            # shared 2-bank tag (banks 6-7) for qk/v/av/rank1 tiles.
            with (
                tc.tile_pool(name="spool", bufs=3, space="PSUM") as sps,
                tc.tile_pool(name="qa", bufs=2, space="PSUM") as qaps,
                tc.tile_pool(name="ptpool", bufs=3) as ptp,
                tc.tile_pool(name="lp", bufs=2) as lp,
                tc.tile_pool(name="ldram", bufs=1, space="DRAM") as ldram,
            ):
                slabs_of = {}
                lrows_of = {p: {} for p in range(6)}

                def emit_qk(ci):
                    dst = qT[ci] if ci < 6 else kT[ci - 6]
                    for ti in range(2):
                        ps = qaps.tile([128, 512], F32, tag="qa", name=f"qk{ci}_{ti}")
                        for c in range(6):
                            nc.tensor.matmul(
                                ps,
                                w_attn_bf[c][:, 128 * ci: 128 * (ci + 1)],
                                xT[c][:, 512 * ti: 512 * (ti + 1)],
                                start=(c == 0),
                                stop=(c == 5),
                            )
                        nc.scalar.activation(
                            dst[:, 512 * ti: 512 * (ti + 1)],
                            ps,
                            AF.Identity,
                            bias=b_attn_col[:, ci: ci + 1],
                        )

                def emit_v(tis):
                    for ti in tis:
                        for ni, (n0, n) in enumerate(((0, 512), (512, 256))):
                            ps = qaps.tile([128, 512], F32, tag="qa", name=f"v{ti}_{ni}")
                            for c in range(6):
                                nc.tensor.matmul(
                                    ps[:, :n],
                                    xT[c][:, 128 * ti: 128 * (ti + 1)],
                                    w_attn_bf[c][:, 2 * C + n0: 2 * C + n0 + n],
                                    start=(c == 0),
                                    stop=False,
                                )
                            nc.tensor.matmul(
                                ps[:, :n],
                                ones_row,
                                b_attn_bf[:, 2 * C + n0: 2 * C + n0 + n],
                                start=False,
                                stop=True,
                            )
                            hn = n // HD
                            nc.scalar.copy(
                                v_aug[ti][:, 8 * ni: 8 * ni + hn, 0:HD],
                                ps[:, :n].rearrange("p (h d) -> p h d", d=HD),
                            )
                        nc.vector.memset(v_aug[ti][:, :, HD: HD + 1], 1.0)
                        nc.vector.memset(v_aug[ti][:, :, HD + 1: 2 * HD], 0.0)

                def emit_av_tile(pn, base, ic):
                    slab = slabs_of[pn]
                    h = 2 * pn + (base // 64)
                    sb = (base // 64) * GW
                    frags = []
                    for jc in range(8):
                        if (jc, ic) in TILE_POS:
                            tr = _trim(jc, ic)
                            for (g, off, w, so) in TILE_POS[(jc, ic)]:
                                frags.append((jc, g, off, w, tr + so))
                    ps = qaps.tile([128, 512], F32, tag="qa", name=f"av{pn}_{base}_{ic}")
                    for k, (jc, g, off, w, o0) in enumerate(frags):
                        nc.tensor.matmul(
                            ps[:, o0: o0 + w],
                            v_aug[jc][:, h, :],
                            slab[:, PW * g + sb + off: PW * g + sb + off + w],
                            start=(k == 0),
                            stop=(k == len(frags) - 1),
                        )
                    nc.vector.tensor_copy(
                        y_pair[pn][base: base + 64, 512 * ic: 512 * (ic + 1)],
                        ps[0:64, :],
                    )
                    lrow = lp.tile([1, 512], F32, tag="lrow", bufs=8)
                    nc.vector.tensor_copy(lrow, ps[64:65, :])
                    lrows_of[pn][(base, ic)] = lrow

                def emit_norm(pn):
                    # 1/l: bounce the l rows through DRAM and broadcast-read
                    # across partitions on the (idle) sync queue, then
                    # exp(-ln l) on ScalarE. Off the critical path (deferred
                    # two pairs behind the AV that produced l).
                    lrows = lrows_of[pn]
                    for ic in (0, 1):
                        rk = lp.tile([128, 512], F32, tag="rk", name=f"rk{pn}_{ic}")
                        for base in (0, 64):
                            ld = ldram.tile(
                                [1, 512], F32, tag="ld", bufs=8,
                                name=f"ld{pn}_{ic}_{base}",
                            )
                            nc.sync.dma_start(out=ld, in_=lrows[(base, ic)])
                            nc.sync.dma_start(
                                out=rk[base: base + 64, :],
                                in_=ld.to_broadcast([64, 512]),
                            )
                        rlog = lp.tile([128, 512], F32, tag="rlog")
                        nc.scalar.activation(rlog, rk, AF.Ln)
                        rpair = lp.tile([128, 512], F32, tag="rpair")
                        nc.scalar.activation(rpair, rlog, AF.Exp, scale=-1.0)
                        nc.vector.tensor_mul(
                            y_pair[pn][:, 512 * ic: 512 * (ic + 1)],
                            y_pair[pn][:, 512 * ic: 512 * (ic + 1)],
                            rpair,
                        )

                AV_AFTER = {2: (0, 0), 4: (0, 1), 6: (64, 0), 8: (64, 1)}
                for p in range(6):
                    emit_qk(p)
                    emit_qk(p + 6)
                    if p == 1:
                        emit_v(range(0, 4))
                    elif p == 2:
                        emit_v(range(4, 8))
                    slab = ptp.tile(
                        [128, SLABW], BF16, tag="slab", name=f"slab{p}", bufs=3
                    )
                    slabs_of[p] = slab
                    sl18 = slab.rearrange("p (a r) -> p a r", a=18)
                    for g, grp in enumerate(GROUPS_LAYOUT):
                        pss = sps.tile([128, PW], F32, tag="spair", name=f"sp{p}_{g}")
                        for (jc, ic, off, w, so) in grp:
                            tr = _trim(jc, ic)
                            q0 = 512 * ic + tr + so
                            for base in (0, 64):
                                nc.tensor.matmul(
                                    pss[:, (base // 64) * GW + off:
                                        (base // 64) * GW + off + w],
                                    kT[p][base: base + 64, 128 * jc: 128 * (jc + 1)],
                                    qT[p][base: base + 64, q0: q0 + w],
                                    start=True,
                                    stop=True,
                                )
                        nc.scalar.activation(
                            slab[:, PW * g: PW * (g + 1)], pss, AF.Exp, scale=SCALE
                        )
                        if g == 0:
                            # g0 (0,0) triangle: blocks 0,1 @0
                            sl = sl18[:, 0:2, 0:128]
                            nc.vector.tensor_mul(
                                sl, sl,
                                m0.rearrange("p (a c) -> p a c", a=1)
                                .to_broadcast([128, 2, 128]),
                            )
                        if p >= 2 and g in AV_AFTER:
                            emit_av_tile(p - 2, *AV_AFTER[g])
                        if p == 5 and g in (1, 3, 5, 7):
                            # block 5 also hosts pair 4's AV to shrink the tail
                            emit_av_tile(4, *AV_AFTER[g + 1])
                    # remaining causal triangles, batched (slab viewed as
                    # 18 blocks of 512: block 2g+half):
                    # @0 blocks 10-17: g5 (4,1), g6 (1,0), g7 (5,1), g8 (2,0)
                    sl = sl18[:, 10:18, 0:128]
                    nc.vector.tensor_mul(
                        sl, sl,
                        m0.rearrange("p (a c) -> p a c", a=1)
                        .to_broadcast([128, 8, 128]),
                    )
                    # @384 blocks 12-15: g6 (3,0), g7 (7,1)
                    sl = sl18[:, 12:16, 384:512]
                    nc.vector.tensor_mul(
                        sl, sl,
                        m0.rearrange("p (a c) -> p a c", a=1)
                        .to_broadcast([128, 4, 128]),
                    )
                    # @256 blocks 16,17: g8 (6,1)
                    sl = sl18[:, 16:18, 256:384]
                    nc.vector.tensor_mul(
                        sl, sl,
                        m0.rearrange("p (a c) -> p a c", a=1)
                        .to_broadcast([128, 2, 128]),
                    )
                    if p >= 2:
                        emit_norm(p - 2)
                emit_norm(4)
                for (base, ic) in ((0, 0), (0, 1), (64, 0), (64, 1)):
                    emit_av_tile(5, base, ic)
                emit_norm(5)

            # ---- phase 3: output projection, two 8-bank waves. Each wave
            # prefills ci 0-4 for all its PSUM groups first, so those
            # matmuls run during the pair-5 AV/normalization chain instead
            # of head-of-line blocking on the norm-gated ci=5.
            with (
                tc.tile_pool(name="pps", bufs=8, space="PSUM") as pps,
                tc.tile_pool(name="ops", bufs=3) as ops,
            ):
                for wave in range(2):
                    tis = range(4 * wave, 4 * wave + 4)
                    pss = {}
                    for ti in tis:
                        for ni, (n0, n) in enumerate(((0, 512), (512, 256))):
                            ps = pps.tile(
                                [128, 512], F32, tag="pp", name=f"pp{ti}_{ni}"
                            )
                            pss[(ti, ni)] = ps
                            for ci in range(5):
                                nc.tensor.matmul(
                                    ps[:, :n],
                                    y_pair[ci][:, 128 * ti: 128 * (ti + 1)],
                                    w_proj_bf[ci][:, n0: n0 + n],
                                    start=(ci == 0),
                                    stop=False,
                                )
                    for ti in tis:
                        osb = ops.tile([128, C], F32, tag="osb", name=f"osb{ti}")
                        for ni, (n0, n) in enumerate(((0, 512), (512, 256))):
                            ps = pss[(ti, ni)]
                            nc.tensor.matmul(
                                ps[:, :n],
                                y_pair[5][:, 128 * ti: 128 * (ti + 1)],
                                w_proj_bf[5][:, n0: n0 + n],
                                start=False,
                                stop=False,
                            )
                            nc.tensor.matmul(
                                ps[:, :n],
                                ones_row,
                                b_proj_bf[:, n0: n0 + n],
                                start=False,
                                stop=True,
                            )
                            # alternate engines: scalar carries norm(5) in
                            # this window, vector is otherwise idle
                            if ti % 2 == 0:
                                nc.scalar.copy(osb[:, n0: n0 + n], ps[:, :n])
                            else:
                                nc.vector.tensor_copy(osb[:, n0: n0 + n], ps[:, :n])
                        nc.sync.dma_start(
                            out=OUT.ap()[128 * ti: 128 * (ti + 1), :], in_=osb
                        )

    return nc


_NC = None
LAST_EXEC_NS = None
LAST_TRACE = None


def _install_ntff_hook_shim():
    """The agent image's antenv lacks axon_hooks; recreate the NTFF
    profiling hook (ctypes into libaxon_pjrt.so) and register a stub
    antenv.axon_hooks module so bass_utils' trace=True path finds it."""
    import sys
    import types
    import ctypes
    import contextlib

    if "antenv.axon_hooks" in sys.modules:
        return
    so_path = "/opt/axon/libaxon_pjrt.so"
    lib = ctypes.CDLL(so_path)
    if not hasattr(lib, "axon_start_nrt_profile"):
        return
    lib.axon_start_nrt_profile.argtypes = [
        ctypes.POINTER(ctypes.c_int64),
        ctypes.c_size_t,
    ]
    lib.axon_start_nrt_profile.restype = ctypes.c_int64
    lib.axon_stop_nrt_profile.argtypes = [ctypes.c_char_p]
    lib.axon_stop_nrt_profile.restype = ctypes.c_int64

    @contextlib.contextmanager
    def _hook(output_dir, device_ids):
        import jax

        jax.devices()
        if device_ids:
            ids = (ctypes.c_int64 * len(device_ids))(*device_ids)
            rc = lib.axon_start_nrt_profile(ids, len(device_ids))
        else:
            rc = lib.axon_start_nrt_profile(None, 0)
        if rc != 0:
            raise RuntimeError(f"axon_start_nrt_profile rc={rc}")
        try:
            yield
        finally:
            n = lib.axon_stop_nrt_profile(str(output_dir).encode())
            if n < 0:
                raise RuntimeError(f"axon_stop_nrt_profile rc={n}")
            print(f"profile: {n} file(s) written to {output_dir}")

    mod = types.ModuleType("antenv.axon_hooks")
    mod.get_axon_ntff_profile_hook = lambda: _hook
    mod.set_axon_ntff_profile_hook = lambda h: None
    sys.modules["antenv.axon_hooks"] = mod


def _get_nc():
    global _NC
    if _NC is None:
        _NC = build_nc()
    return _NC


def kernel(x, W_attn, b_attn, W_proj, b_proj):
    global LAST_EXEC_NS, LAST_TRACE
    x = np.ascontiguousarray(np.asarray(x, dtype=np.float32))
    W_attn = np.ascontiguousarray(np.asarray(W_attn, dtype=np.float32))
    b_attn = np.ascontiguousarray(np.asarray(b_attn, dtype=np.float32))
    W_proj = np.ascontiguousarray(np.asarray(W_proj, dtype=np.float32))
    b_proj = np.ascontiguousarray(np.asarray(b_proj, dtype=np.float32))

    nc = _get_nc()
    in_maps = [
        {
            "x": x[b],
            "W_attn": W_attn,
            "b_attn": b_attn,
            "W_proj": W_proj,
            "b_proj": b_proj,
        }
        for b in range(B)
    ]
    import os

    trace = bool(os.environ.get("KERNEL_TRACE"))
    if trace:
        _install_ntff_hook_shim()
    res = run_bass_kernel_spmd(
        nc, in_maps, core_ids=list(range(B)), trace=trace
    )
    if res.exec_time_ns is not None:
        LAST_EXEC_NS = res.exec_time_ns
    if res.instructions_and_trace is not None:
        LAST_TRACE = res.instructions_and_trace[1]
    return np.stack([r["out"] for r in res.results], axis=0)


if __name__ == "__main__":
    rng = np.random.default_rng(0)
    inputs = {
        "x": rng.standard_normal((B, T, C), dtype=np.float32),
        "W_attn": (rng.standard_normal((C, C3), dtype=np.float32) * 0.02),
        "b_attn": np.zeros((C3,), np.float32),
        "W_proj": (rng.standard_normal((C, C), dtype=np.float32) * 0.02),
        "b_proj": np.zeros((C,), np.float32),
    }
    out = kernel(**inputs)
    print("out shape", out.shape, out.dtype)
